# revision 32
# baseline (speedup 1.0000x reference)
"""Minibatch discrimination kernel for Trainium2 (8 NeuronCores, Bass/Tile).

Reference computation:
    M = (x @ T.reshape(1024, 2048)).reshape(256, 128, 16)
    norm[i, j, o] = sum_k |M[j, o, k] - M[i, o, k]|
    o_b = exp(-norm).sum(0) - 1            # [256, 128]
    out = concat([x, o_b], axis=1)         # [256, 1152]

Sharding: each of the 8 cores owns 16 of the 128 output features o.  Every
core receives the full x^T and its T[:, o_slice, :] slice; there are no
collectives.  Per core, with ok = (o_local, k) flattened to 256 (2 partition
blocks of 128):

  1. PE computes Mt[ok, i] = sum_f T[f, ok] * x[i, f]  (bf16 in, fp32 psum).
  2. Loop over 32 chunks g of 8 batch rows i.  Exploiting norm symmetry, chunk
     g only computes partners j >= 8g.  Using |a-b| = a + b - 2*min(a,b):
       - DVE/GPSIMD tensor_scalar (op0=min, op1=subtract, both scalars
         per-partition columns of an fp32 image of Mt) produces
         t = min(Mt[:,j], m_i) - m_i/2 at the 4x bf16 rate, per (i, block).
       - PE builds psum = norm directly: one [128,128]-selector matmul per
         block adds S_j = sum_k Mt[ok, j] into all four 32-row regions at
         once, and per-(i, block) selector matmuls with entries -2 add
         -2*sum_k t.  Matmul outputs must start at 32-aligned psum
         partitions, so slot s lives at rows (s%4)*32 + o in one of two psum
         tiles (slots 0-3 / 4-7).
       - ScalarE exp(-psum) with accum_out gives the row sums directly.
       - PE accumulates column sums of the exp block (for j in later chunks)
         into a persistent psum accumulator -> the mirrored lower triangle.
  3. Host combines row sums + column sums - 1 and concatenates with x.

The self-term norm[i,i] is exactly 0 on hardware: the per-partition scalars
are bit-exact fp32 images of the bf16 operand, min(a,a)-a/2 = a/2 is exact in
bf16, the -2 selector entries make the t-matmul products exactly -Mt[ok,i],
and the S_j and t matmuls accumulate the same values over the same partition
sequence with opposite signs (round-to-nearest is sign-symmetric), so
exp(-psum) = 1 and the host's -1 yields an exact 0, matching the reference
(whose o_b underflows to exactly 0 everywhere for this problem's data).
"""

import numpy as np
import ml_dtypes

B = 256
IN_F = 1024
OUT_F = 128
KD = 16
N_CORES = 8
O_PER_CORE = OUT_F // N_CORES      # 16
OK = O_PER_CORE * KD               # 256 = 2 partition blocks of 128
O_PER_BLOCK = 128 // KD            # 8 o's per 128-partition block
IB = 8                             # batch rows per chunk
NG = B // IB                       # 32 chunks

_BF16 = ml_dtypes.bfloat16

_CACHE: dict = {}


def _build_program():
    import concourse.bacc as bacc
    import concourse.bass as bass
    import concourse.mybir as mybir
    import concourse.tile as tile

    f32 = mybir.dt.float32
    bf16 = mybir.dt.bfloat16
    AF = mybir.ActivationFunctionType
    ALU = mybir.AluOpType

    nc = bacc.Bacc("TRN2", target_bir_lowering=False, debug=False,
                   num_devices=N_CORES)

    xt_d = nc.dram_tensor("xt", [IN_F, B], bf16, kind="ExternalInput")
    tm_d = nc.dram_tensor("tmat", [IN_F, OK], bf16, kind="ExternalInput")
    sel32_d = [
        nc.dram_tensor(f"sel32_{b}", [128, 32], bf16, kind="ExternalInput")
        for b in range(2)
    ]
    selB_d = [
        nc.dram_tensor(f"selB_{b}", [128, 128], bf16, kind="ExternalInput")
        for b in range(2)
    ]
    sel32p_d = [
        nc.dram_tensor(f"sel32p_{b}", [128, 32], bf16, kind="ExternalInput")
        for b in range(2)
    ]
    selS_d = [
        nc.dram_tensor(f"selS_{b}", [128, O_PER_CORE], bf16,
                       kind="ExternalInput")
        for b in range(2)
    ]
    selc_d = nc.dram_tensor("selc", [128, O_PER_CORE], bf16,
                            kind="ExternalInput")
    rows_d = [
        nc.dram_tensor(f"rows{h}", [128, NG], f32, kind="ExternalOutput")
        for h in range(2)
    ]
    cols_d = nc.dram_tensor("cols", [O_PER_CORE, B], f32,
                            kind="ExternalOutput")

    with tile.TileContext(nc) as tc:
        with (
            tc.tile_pool(name="cst", bufs=1) as cst,
            tc.tile_pool(name="big", bufs=1) as big,
            tc.tile_pool(name="mt", bufs=1) as mtp,
            tc.tile_pool(name="tmp", bufs=32) as tmpp,
            tc.tile_pool(name="ee", bufs=4) as ep,
            tc.tile_pool(name="res", bufs=1) as resp,
            tc.tile_pool(name="pp", bufs=1, space=bass.MemorySpace.PSUM) as pp,
            tc.tile_pool(name="pn", bufs=3, space=bass.MemorySpace.PSUM) as pn,
            tc.tile_pool(name="pc", bufs=1, space=bass.MemorySpace.PSUM) as pc,
        ):
            # ---------- load inputs ----------
            xt_sb = big.tile([128, IN_F // 128, B], bf16, tag="xt")
            tm_sb = big.tile([128, IN_F // 128, OK], bf16, tag="tm")
            sel32_sb = [cst.tile([128, 32], bf16, tag=f"sel32_{b}", name=f"sel32_{b}")
                        for b in range(2)]
            selB_sb = [cst.tile([128, 128], bf16, tag=f"selB_{b}",
                                name=f"selB_{b}")
                       for b in range(2)]
            sel32p_sb = [cst.tile([128, 32], bf16, tag=f"sel32p_{b}",
                                  name=f"sel32p_{b}")
                         for b in range(2)]
            selS_sb = [cst.tile([128, O_PER_CORE], bf16, tag=f"selS_{b}",
                                name=f"selS_{b}")
                       for b in range(2)]
            selc_sb = cst.tile([128, O_PER_CORE], bf16, tag="selc")
            xt_r = xt_d[:].rearrange("(c p) i -> p c i", p=128)
            tm_r = tm_d[:].rearrange("(c p) o -> p c o", p=128)
            nc.sync.dma_start(xt_sb[:, :4, :], xt_r[:, :4, :])
            nc.scalar.dma_start(xt_sb[:, 4:, :], xt_r[:, 4:, :])
            nc.gpsimd.dma_start(tm_sb[:, :4, :], tm_r[:, :4, :])
            nc.gpsimd.dma_start(tm_sb[:, 4:, :], tm_r[:, 4:, :])
            for b in range(2):
                nc.gpsimd.dma_start(sel32_sb[b][:], sel32_d[b][:])
                nc.gpsimd.dma_start(selB_sb[b][:], selB_d[b][:])
                nc.gpsimd.dma_start(sel32p_sb[b][:], sel32p_d[b][:])
                nc.gpsimd.dma_start(selS_sb[b][:], selS_d[b][:])
            nc.gpsimd.dma_start(selc_sb[:], selc_d[:])

            # ---------- projection: Mt[ok, i] ----------
            # mtb: bf16 working copy; mt32: exact fp32 upcast of the SAME
            # bf16 values so the per-partition scalar (must be fp32) matches
            # the streamed operand bit-for-bit -> norm[i,i] == 0 exactly.
            mtb = []
            mt32 = []
            mt32h = []
            for b in range(2):
                pm = pp.tile([128, 512], f32, tag="pm")
                for c in range(IN_F // 128):
                    nc.tensor.matmul(
                        pm[:, :B],
                        tm_sb[:, c, b * 128:(b + 1) * 128],
                        xt_sb[:, c, :],
                        start=(c == 0),
                        stop=(c == IN_F // 128 - 1),
                    )
                mb = mtp.tile([128, B], bf16, tag=f"mtb{b}", name=f"mtb{b}")
                m3 = mtp.tile([128, B], f32, tag=f"mt32{b}", name=f"mt32{b}")
                mh = mtp.tile([128, B], f32, tag=f"mt32h{b}", name=f"mt32h{b}")
                nc.vector.tensor_copy(mb[:], pm[:, :B])
                nc.vector.tensor_copy(m3[:], mb[:])
                # exact fp32 halves of the bf16 Mt values
                nc.vector.tensor_scalar(mh[:], mb[:], 0.5, None, op0=ALU.mult)
                mtb.append(mb)
                mt32.append(m3)
                mt32h.append(mh)

            # S[o, i] = sum_k Mt[(o,k), i]; slot s=1 is computed on ACT as
            # relu(m_i - a), whose psum is norm + S_i, corrected by an exp
            # bias of +S_i gathered into rows 32:48 of bias_all per chunk.
            ps = pp.tile([O_PER_CORE, 512], f32, tag="pm", name="ps")
            for b in range(2):
                nc.tensor.matmul(ps[:, :B], selS_sb[b][:], mtb[b][:],
                                 start=(b == 0), stop=(b == 1))
            s_sb = resp.tile([O_PER_CORE, B], f32, tag="s_sb", name="s_sb")
            nc.vector.tensor_copy(s_sb[:], ps[:, :B])
            bias_all = resp.tile([128, NG], f32, tag="bias_all",
                                 name="bias_all")
            nc.vector.memset(bias_all[:], 0.0)
            nc.sync.dma_start(
                bias_all[32:48, :].rearrange("o (g u) -> o g u", u=1),
                s_sb[:].rearrange("o (g q) -> o g q", q=IB)[:, :, 1:2],
            )

            # ---------- main loop over row chunks ----------
            rows_sb = [resp.tile([128, NG], f32, tag=f"rows{h}", name=f"rows{h}_sb")
                       for h in range(2)]
            pcol = pc.tile([O_PER_CORE, 512], f32, tag="pcol")

            for g in range(NG):
                jlo = IB * g
                jw = B - jlo
                pnorm = [pn.tile([128, 512], f32, tag=f"pnorm{h}",
                                  name=f"pnorm{h}")
                         for h in range(2)]
                # psum = S_j - 2*sum_k t = norm, with t = min(Mt_j, m_i) - m_i/2:
                # the S_j term comes from one [128,128]-selector matmul per
                # block that fills all four 32-row regions at once, and the
                # t-matmuls (selector entries -2) accumulate on top.
                for h in range(2):
                    for b in range(2):
                        nc.tensor.matmul(
                            pnorm[h][:, :jw],
                            selB_sb[b][:],
                            mtb[b][:, jlo:B],
                            start=(b == 0),
                            stop=False,
                            skip_group_check=True,
                        )
                # GPSIMD takes a share of the abs-ops; its per-op cost is
                # ~linear in jw while DVE is mostly fixed overhead, so the
                # optimal share grows as jw shrinks.
                if g < 2:
                    gps = ((0, 1), (4, 0))
                elif g < 8:
                    gps = ((0, 0), (0, 1), (4, 0), (4, 1))
                elif g < 16:
                    gps = ((0, 0), (0, 1), (2, 0), (4, 0), (4, 1))
                elif g < 28:
                    gps = ((0, 0), (0, 1), (2, 0), (4, 0), (4, 1), (6, 1))
                else:
                    gps = ((0, 0), (0, 1), (2, 0), (3, 1), (4, 0), (4, 1),
                           (6, 1))
                for s in range(IB):
                    i = jlo + s
                    h, r0 = divmod(s, 4)
                    if s == 1:
                        # ACT path: r = relu(m_i - a); psum += 2*sum_k r
                        for b in range(2):
                            rt = tmpp.tile([128, jw], bf16, tag="tmp",
                                           name="rt")
                            nc.scalar.activation(
                                rt[:],
                                mtb[b][:, jlo:B],
                                AF.Relu,
                                scale=-1.0,
                                bias=mt32[b][:, i:i + 1],
                            )
                            nc.tensor.matmul(
                                pnorm[h][r0 * 32: r0 * 32 + 32, :jw],
                                sel32p_sb[b][:],
                                rt[:],
                                start=False,
                                stop=False,
                                tile_position=(0, r0 * 32),
                                skip_group_check=True,
                            )
                        continue
                    for b in range(2):
                        t = tmpp.tile([128, jw], bf16, tag="tmp", name="t")
                        teng = nc.gpsimd if (s, b) in gps else nc.vector
                        teng.tensor_scalar(
                            t[:],
                            mtb[b][:, jlo:B],
                            mt32[b][:, i:i + 1],
                            mt32h[b][:, i:i + 1],
                            op0=ALU.min,
                            op1=ALU.subtract,
                        )
                        nc.tensor.matmul(
                            pnorm[h][r0 * 32: r0 * 32 + 32, :jw],
                            sel32_sb[b][:],
                            t[:],
                            start=False,
                            stop=(r0 == 3 and b == 1),
                            tile_position=(0, r0 * 32),
                            skip_group_check=True,
                        )
                ee = []
                for h in range(2):
                    e = ep.tile([128, jw], bf16, tag=f"e{h}", name=f"e{h}")
                    nc.scalar.activation(
                        e[:],
                        pnorm[h][:, :jw],
                        AF.Exp,
                        scale=-1.0,
                        bias=bias_all[:, g:g + 1] if h == 0 else 0.0,
                        accum_out=rows_sb[h][:, g:g + 1],
                    )
                    ee.append(e)
                if g < NG - 1:
                    for h in range(2):
                        nc.tensor.matmul(
                            pcol[:, jlo + IB:B],
                            selc_sb[:],
                            ee[h][:, IB:],
                            start=(g == 0 and h == 0),
                            stop=(g == NG - 2 and h == 1),
                            skip_group_check=True,
                        )
                if g == 15:
                    # first 16 row-sum columns are final; drain them early
                    nc.sync.dma_start(rows_d[0][:, :16], rows_sb[0][:, :16])
                    nc.scalar.dma_start(rows_d[1][:, :16], rows_sb[1][:, :16])

            # ---------- finalize ----------
            cols_sb = resp.tile([O_PER_CORE, B], f32, tag="cols")
            nc.vector.memset(cols_sb[:, 0:IB], 0.0)
            nc.vector.tensor_copy(cols_sb[:, IB:B], pcol[:, IB:B])
            nc.sync.dma_start(rows_d[0][:, 16:], rows_sb[0][:, 16:])
            nc.scalar.dma_start(rows_d[1][:, 16:], rows_sb[1][:, 16:])
            nc.gpsimd.dma_start(cols_d[:], cols_sb[:])

    nc.compile()
    return nc


def get_program():
    if "nc" not in _CACHE:
        _CACHE["nc"] = _build_program()
    return _CACHE["nc"]


def make_selectors():
    p = np.arange(128)
    sel32 = []
    sel32p = []
    selB = []
    selS = []
    for b in range(2):
        s = np.zeros((128, 32), dtype=_BF16)
        s[p, p // KD + b * O_PER_BLOCK] = -2   # cols b*8..b*8+8 active
        sel32.append(s)
        sp = np.zeros((128, 32), dtype=_BF16)
        sp[p, p // KD + b * O_PER_BLOCK] = 2
        sel32p.append(sp)
        sb = np.zeros((128, 128), dtype=_BF16)
        for r0 in range(4):
            sb[p, r0 * 32 + p // KD + b * O_PER_BLOCK] = 1
        selB.append(sb)
        ss = np.zeros((128, O_PER_CORE), dtype=_BF16)
        ss[p, p // KD + b * O_PER_BLOCK] = 1
        selS.append(ss)
    selc = np.zeros((128, O_PER_CORE), dtype=_BF16)
    selc[p % 32 < 16, p[p % 32 < 16] % 32] = 1
    return sel32, sel32p, selB, selS, selc


def prepare_in_maps(x: np.ndarray, T: np.ndarray) -> list[dict[str, np.ndarray]]:
    x = np.asarray(x, dtype=np.float32)
    T = np.asarray(T, dtype=np.float32)
    xt = np.ascontiguousarray(x.T).astype(_BF16)
    sel32, sel32p, selB, selS, selc = make_selectors()
    in_maps = []
    for c in range(N_CORES):
        tm = np.ascontiguousarray(
            T[:, c * O_PER_CORE:(c + 1) * O_PER_CORE, :].reshape(IN_F, OK)
        ).astype(_BF16)
        in_maps.append({
            "xt": xt, "tmat": tm,
            "sel32_0": sel32[0], "sel32_1": sel32[1],
            "sel32p_0": sel32p[0], "sel32p_1": sel32p[1],
            "selB_0": selB[0], "selB_1": selB[1],
            "selS_0": selS[0], "selS_1": selS[1], "selc": selc,
        })
    return in_maps


def assemble_output(x: np.ndarray,
                    results: list[dict[str, np.ndarray]]) -> np.ndarray:
    x = np.asarray(x, dtype=np.float32)
    o_b = np.empty((B, OUT_F), dtype=np.float32)
    for c in range(N_CORES):
        cols = np.asarray(results[c]["cols"], dtype=np.float32)  # [16, B]
        R = np.empty((B, O_PER_CORE), dtype=np.float32)
        for h in range(2):
            rows = np.asarray(results[c][f"rows{h}"], dtype=np.float32)
            # rows[(s%4)*32 + o, g] -> R[i = g*8 + h*4 + s%4, o]
            r4 = rows.reshape(4, 32, NG)[:, :O_PER_CORE, :]   # [s4, o, g]
            idx = (np.arange(NG)[:, None] * IB + h * 4 +
                   np.arange(4)[None, :]).reshape(-1)
            R[idx, :] = r4.transpose(2, 0, 1).reshape(NG * 4, O_PER_CORE)
        o_b[:, c * O_PER_CORE:(c + 1) * O_PER_CORE] = R + cols.T - 1.0
    return np.concatenate([x, o_b], axis=1)


def kernel(x: np.ndarray, T: np.ndarray) -> np.ndarray:
    from concourse.bass_utils import run_bass_kernel_spmd

    nc = get_program()
    in_maps = prepare_in_maps(x, T)
    res = run_bass_kernel_spmd(nc, in_maps, list(range(N_CORES)))
    return assemble_output(x, res.results)


# revision 33
# speedup vs baseline: 1.3945x; 1.3945x over previous
"""Minibatch discrimination kernel for Trainium2 (8 NeuronCores, Bass/Tile).

Reference computation:
    M = (x @ T.reshape(1024, 2048)).reshape(256, 128, 16)
    norm[i, j, o] = sum_k |M[j, o, k] - M[i, o, k]|
    o_b = exp(-norm).sum(0) - 1            # [256, 128]
    out = concat([x, o_b], axis=1)         # [256, 1152]

Sharding: each of the 8 cores owns 16 of the 128 output features o.  Every
core receives the full x^T and its T[:, o_slice, :] slice; there are no
collectives.  Per core, with ok = (o_local, k) flattened to 256 (2 partition
blocks of 128):

  1. PE computes Mt[ok, i] = sum_f T[f, ok] * x[i, f]  (bf16 in, fp32 psum).
  2. Loop over 32 chunks g of 8 batch rows i.  Exploiting norm symmetry, chunk
     g only computes partners j >= 8g.  Using |a-b| = a + b - 2*min(a,b):
       - DVE/GPSIMD tensor_scalar (op0=min, op1=subtract, both scalars
         per-partition columns of an fp32 image of Mt) produces
         t = min(Mt[:,j], m_i) - m_i/2 at the 4x bf16 rate, per (i, block).
       - PE builds psum = norm directly: one [128,128]-selector matmul per
         block adds S_j = sum_k Mt[ok, j] into all four 32-row regions at
         once, and per-(i, block) selector matmuls with entries -2 add
         -2*sum_k t.  Matmul outputs must start at 32-aligned psum
         partitions, so slot s lives at rows (s%4)*32 + o in one of two psum
         tiles (slots 0-3 / 4-7).
       - ScalarE exp(-psum) with accum_out gives the row sums directly.
       - PE accumulates column sums of the exp block (for j in later chunks)
         into a persistent psum accumulator -> the mirrored lower triangle.
  3. Host combines row sums + column sums - 1 and concatenates with x.

The self-term norm[i,i] is exactly 0 on hardware: the per-partition scalars
are bit-exact fp32 images of the bf16 operand, min(a,a)-a/2 = a/2 is exact in
bf16, the -2 selector entries make the t-matmul products exactly -Mt[ok,i],
and the S_j and t matmuls accumulate the same values over the same partition
sequence with opposite signs (round-to-nearest is sign-symmetric), so
exp(-psum) = 1 and the host's -1 yields an exact 0, matching the reference
(whose o_b underflows to exactly 0 everywhere for this problem's data).
"""

import numpy as np
import ml_dtypes

B = 256
IN_F = 1024
OUT_F = 128
KD = 16
N_CORES = 8
O_PER_CORE = OUT_F // N_CORES      # 16
OK = O_PER_CORE * KD               # 256 = 2 partition blocks of 128
O_PER_BLOCK = 128 // KD            # 8 o's per 128-partition block
IB = 8                             # batch rows per chunk
NG = B // IB                       # 32 chunks

_BF16 = ml_dtypes.bfloat16

_CACHE: dict = {}


def _build_program():
    import concourse.bacc as bacc
    import concourse.bass as bass
    import concourse.mybir as mybir
    import concourse.tile as tile

    f32 = mybir.dt.float32
    bf16 = mybir.dt.bfloat16
    AF = mybir.ActivationFunctionType
    ALU = mybir.AluOpType

    nc = bacc.Bacc("TRN2", target_bir_lowering=False, debug=False,
                   num_devices=N_CORES)

    xt_d = nc.dram_tensor("xt", [IN_F, B], bf16, kind="ExternalInput")
    tm_d = nc.dram_tensor("tmat", [IN_F, OK], bf16, kind="ExternalInput")
    sel32_d = [
        nc.dram_tensor(f"sel32_{b}", [128, 32], bf16, kind="ExternalInput")
        for b in range(2)
    ]
    selB_d = [
        nc.dram_tensor(f"selB_{b}", [128, 128], bf16, kind="ExternalInput")
        for b in range(2)
    ]
    selc_d = nc.dram_tensor("selc", [128, O_PER_CORE], bf16,
                            kind="ExternalInput")
    rows_d = [
        nc.dram_tensor(f"rows{h}", [128, NG], f32, kind="ExternalOutput")
        for h in range(2)
    ]
    cols_d = nc.dram_tensor("cols", [O_PER_CORE, B], f32,
                            kind="ExternalOutput")

    with tile.TileContext(nc) as tc:
        with (
            tc.tile_pool(name="cst", bufs=1) as cst,
            tc.tile_pool(name="big", bufs=1) as big,
            tc.tile_pool(name="mt", bufs=1) as mtp,
            tc.tile_pool(name="tmp", bufs=32) as tmpp,
            tc.tile_pool(name="ee", bufs=4) as ep,
            tc.tile_pool(name="res", bufs=1) as resp,
            tc.tile_pool(name="pp", bufs=1, space=bass.MemorySpace.PSUM) as pp,
            tc.tile_pool(name="pn", bufs=3, space=bass.MemorySpace.PSUM) as pn,
            tc.tile_pool(name="pc", bufs=1, space=bass.MemorySpace.PSUM) as pc,
        ):
            # ---------- load inputs ----------
            xt_sb = big.tile([128, IN_F // 128, B], bf16, tag="xt")
            tm_sb = big.tile([128, IN_F // 128, OK], bf16, tag="tm")
            sel32_sb = [cst.tile([128, 32], bf16, tag=f"sel32_{b}", name=f"sel32_{b}")
                        for b in range(2)]
            selB_sb = [cst.tile([128, 128], bf16, tag=f"selB_{b}",
                                name=f"selB_{b}")
                       for b in range(2)]
            selc_sb = cst.tile([128, O_PER_CORE], bf16, tag="selc")
            xt_r = xt_d[:].rearrange("(c p) i -> p c i", p=128)
            tm_r = tm_d[:].rearrange("(c p) o -> p c o", p=128)
            nc.sync.dma_start(xt_sb[:, :4, :], xt_r[:, :4, :])
            nc.scalar.dma_start(xt_sb[:, 4:, :], xt_r[:, 4:, :])
            nc.gpsimd.dma_start(tm_sb[:, :4, :], tm_r[:, :4, :])
            nc.gpsimd.dma_start(tm_sb[:, 4:, :], tm_r[:, 4:, :])
            for b in range(2):
                nc.gpsimd.dma_start(sel32_sb[b][:], sel32_d[b][:])
                nc.gpsimd.dma_start(selB_sb[b][:], selB_d[b][:])
            nc.gpsimd.dma_start(selc_sb[:], selc_d[:])

            # ---------- projection: Mt[ok, i] ----------
            # mtb: bf16 working copy; mt32: exact fp32 upcast of the SAME
            # bf16 values so the per-partition scalar (must be fp32) matches
            # the streamed operand bit-for-bit -> norm[i,i] == 0 exactly.
            mtb = []
            mt32 = []
            mt32h = []
            for b in range(2):
                pm = pp.tile([128, 512], f32, tag="pm")
                for c in range(IN_F // 128):
                    nc.tensor.matmul(
                        pm[:, :B],
                        tm_sb[:, c, b * 128:(b + 1) * 128],
                        xt_sb[:, c, :],
                        start=(c == 0),
                        stop=(c == IN_F // 128 - 1),
                    )
                mb = mtp.tile([128, B], bf16, tag=f"mtb{b}", name=f"mtb{b}")
                m3 = mtp.tile([128, B], f32, tag=f"mt32{b}", name=f"mt32{b}")
                mh = mtp.tile([128, B], f32, tag=f"mt32h{b}", name=f"mt32h{b}")
                nc.vector.tensor_copy(mb[:], pm[:, :B])
                nc.vector.tensor_copy(m3[:], mb[:])
                # exact fp32 halves of the bf16 Mt values
                nc.vector.tensor_scalar(mh[:], mb[:], 0.5, None, op0=ALU.mult)
                mtb.append(mb)
                mt32.append(m3)
                mt32h.append(mh)

            # ---------- main loop over row chunks ----------
            rows_sb = [resp.tile([128, NG], f32, tag=f"rows{h}", name=f"rows{h}_sb")
                       for h in range(2)]
            pcol = pc.tile([O_PER_CORE, 512], f32, tag="pcol")

            for g in range(NG):
                jlo = IB * g
                jw = B - jlo
                pnorm = [pn.tile([128, 512], f32, tag=f"pnorm{h}",
                                  name=f"pnorm{h}")
                         for h in range(2)]
                # psum = S_j - 2*sum_k t = norm, with t = min(Mt_j, m_i) - m_i/2:
                # the S_j term comes from one [128,128]-selector matmul per
                # block that fills all four 32-row regions at once, and the
                # t-matmuls (selector entries -2) accumulate on top.
                for h in range(2):
                    for b in range(2):
                        nc.tensor.matmul(
                            pnorm[h][:, :jw],
                            selB_sb[b][:],
                            mtb[b][:, jlo:B],
                            start=(b == 0),
                            stop=False,
                            skip_group_check=True,
                        )
                # GPSIMD takes a share of the abs-ops; its per-op cost is
                # ~linear in jw while DVE is mostly fixed overhead, so the
                # optimal share grows as jw shrinks.
                if g < 2:
                    gps = ((0, 1), (4, 0))
                elif g < 8:
                    gps = ((0, 0), (0, 1), (4, 0), (4, 1))
                elif g < 16:
                    gps = ((0, 0), (0, 1), (2, 0), (4, 0), (4, 1))
                elif g < 28:
                    gps = ((0, 0), (0, 1), (2, 0), (4, 0), (4, 1), (6, 1))
                else:
                    gps = ((0, 0), (0, 1), (2, 0), (3, 1), (4, 0), (4, 1),
                           (6, 1))
                for s in range(IB):
                    i = jlo + s
                    h, r0 = divmod(s, 4)
                    for b in range(2):
                        t = tmpp.tile([128, jw], bf16, tag="tmp", name="t")
                        teng = nc.gpsimd if (s, b) in gps else nc.vector
                        teng.tensor_scalar(
                            t[:],
                            mtb[b][:, jlo:B],
                            mt32[b][:, i:i + 1],
                            mt32h[b][:, i:i + 1],
                            op0=ALU.min,
                            op1=ALU.subtract,
                        )
                        nc.tensor.matmul(
                            pnorm[h][r0 * 32: r0 * 32 + 32, :jw],
                            sel32_sb[b][:],
                            t[:],
                            start=False,
                            stop=(r0 == 3 and b == 1),
                            tile_position=(0, r0 * 32),
                            skip_group_check=True,
                        )
                ee = []
                for h in range(2):
                    e = ep.tile([128, jw], bf16, tag=f"e{h}", name=f"e{h}")
                    nc.scalar.activation(
                        e[:],
                        pnorm[h][:, :jw],
                        AF.Exp,
                        scale=-1.0,
                        accum_out=rows_sb[h][:, g:g + 1],
                    )
                    ee.append(e)
                if g < NG - 1:
                    for h in range(2):
                        nc.tensor.matmul(
                            pcol[:, jlo + IB:B],
                            selc_sb[:],
                            ee[h][:, IB:],
                            start=(g == 0 and h == 0),
                            stop=(g == NG - 2 and h == 1),
                            skip_group_check=True,
                        )
                if g == 15:
                    # first 16 row-sum columns are final; drain them early
                    nc.sync.dma_start(rows_d[0][:, :16], rows_sb[0][:, :16])
                    nc.scalar.dma_start(rows_d[1][:, :16], rows_sb[1][:, :16])

            # ---------- finalize ----------
            cols_sb = resp.tile([O_PER_CORE, B], f32, tag="cols")
            nc.vector.memset(cols_sb[:, 0:IB], 0.0)
            nc.vector.tensor_copy(cols_sb[:, IB:B], pcol[:, IB:B])
            nc.sync.dma_start(rows_d[0][:, 16:], rows_sb[0][:, 16:])
            nc.scalar.dma_start(rows_d[1][:, 16:], rows_sb[1][:, 16:])
            nc.gpsimd.dma_start(cols_d[:], cols_sb[:])

    nc.compile()
    return nc


def get_program():
    if "nc" not in _CACHE:
        _CACHE["nc"] = _build_program()
    return _CACHE["nc"]


def make_selectors():
    p = np.arange(128)
    sel32 = []
    selB = []
    for b in range(2):
        s = np.zeros((128, 32), dtype=_BF16)
        s[p, p // KD + b * O_PER_BLOCK] = -2   # cols b*8..b*8+8 active
        sel32.append(s)
        sb = np.zeros((128, 128), dtype=_BF16)
        for r0 in range(4):
            sb[p, r0 * 32 + p // KD + b * O_PER_BLOCK] = 1
        selB.append(sb)
    selc = np.zeros((128, O_PER_CORE), dtype=_BF16)
    selc[p % 32 < 16, p[p % 32 < 16] % 32] = 1
    return sel32, selB, selc


def prepare_in_maps(x: np.ndarray, T: np.ndarray) -> list[dict[str, np.ndarray]]:
    x = np.asarray(x, dtype=np.float32)
    T = np.asarray(T, dtype=np.float32)
    xt = np.ascontiguousarray(x.T).astype(_BF16)
    sel32, selB, selc = make_selectors()
    in_maps = []
    for c in range(N_CORES):
        tm = np.ascontiguousarray(
            T[:, c * O_PER_CORE:(c + 1) * O_PER_CORE, :].reshape(IN_F, OK)
        ).astype(_BF16)
        in_maps.append({
            "xt": xt, "tmat": tm,
            "sel32_0": sel32[0], "sel32_1": sel32[1],
            "selB_0": selB[0], "selB_1": selB[1], "selc": selc,
        })
    return in_maps


def assemble_output(x: np.ndarray,
                    results: list[dict[str, np.ndarray]]) -> np.ndarray:
    x = np.asarray(x, dtype=np.float32)
    o_b = np.empty((B, OUT_F), dtype=np.float32)
    for c in range(N_CORES):
        cols = np.asarray(results[c]["cols"], dtype=np.float32)  # [16, B]
        R = np.empty((B, O_PER_CORE), dtype=np.float32)
        for h in range(2):
            rows = np.asarray(results[c][f"rows{h}"], dtype=np.float32)
            # rows[(s%4)*32 + o, g] -> R[i = g*8 + h*4 + s%4, o]
            r4 = rows.reshape(4, 32, NG)[:, :O_PER_CORE, :]   # [s4, o, g]
            idx = (np.arange(NG)[:, None] * IB + h * 4 +
                   np.arange(4)[None, :]).reshape(-1)
            R[idx, :] = r4.transpose(2, 0, 1).reshape(NG * 4, O_PER_CORE)
        o_b[:, c * O_PER_CORE:(c + 1) * O_PER_CORE] = R + cols.T - 1.0
    return np.concatenate([x, o_b], axis=1)


def kernel(x: np.ndarray, T: np.ndarray) -> np.ndarray:
    from concourse.bass_utils import run_bass_kernel_spmd

    nc = get_program()
    in_maps = prepare_in_maps(x, T)
    res = run_bass_kernel_spmd(nc, in_maps, list(range(N_CORES)))
    return assemble_output(x, res.results)


# revision 34
# speedup vs baseline: 1.3951x; 1.0004x over previous
"""Minibatch discrimination kernel for Trainium2 (8 NeuronCores, Bass/Tile).

Reference computation:
    M = (x @ T.reshape(1024, 2048)).reshape(256, 128, 16)
    norm[i, j, o] = sum_k |M[j, o, k] - M[i, o, k]|
    o_b = exp(-norm).sum(0) - 1            # [256, 128]
    out = concat([x, o_b], axis=1)         # [256, 1152]

Sharding: each of the 8 cores owns 16 of the 128 output features o.  Every
core receives the full x^T and its T[:, o_slice, :] slice; there are no
collectives.  Per core, with ok = (o_local, k) flattened to 256 (2 partition
blocks of 128):

  1. PE computes Mt[ok, i] = sum_f T[f, ok] * x[i, f]  (bf16 in, fp32 psum).
  2. Loop over 32 chunks g of 8 batch rows i.  Exploiting norm symmetry, chunk
     g only computes partners j >= 8g.  Using |a-b| = a + b - 2*min(a,b):
       - DVE/GPSIMD tensor_scalar (op0=min, op1=subtract, both scalars
         per-partition columns of an fp32 image of Mt) produces
         t = min(Mt[:,j], m_i) - m_i/2 at the 4x bf16 rate, per (i, block).
       - PE builds psum = norm directly: one [128,128]-selector matmul per
         block adds S_j = sum_k Mt[ok, j] into all four 32-row regions at
         once, and per-(i, block) selector matmuls with entries -2 add
         -2*sum_k t.  Matmul outputs must start at 32-aligned psum
         partitions, so slot s lives at rows (s%4)*32 + o in one of two psum
         tiles (slots 0-3 / 4-7).
       - ScalarE exp(-psum) with accum_out gives the row sums directly.
       - PE accumulates column sums of the exp block (for j in later chunks)
         into a persistent psum accumulator -> the mirrored lower triangle.
  3. Host combines row sums + column sums - 1 and concatenates with x.

The self-term norm[i,i] is exactly 0 on hardware: the per-partition scalars
are bit-exact fp32 images of the bf16 operand, min(a,a)-a/2 = a/2 is exact in
bf16, the -2 selector entries make the t-matmul products exactly -Mt[ok,i],
and the S_j and t matmuls accumulate the same values over the same partition
sequence with opposite signs (round-to-nearest is sign-symmetric), so
exp(-psum) = 1 and the host's -1 yields an exact 0, matching the reference
(whose o_b underflows to exactly 0 everywhere for this problem's data).
"""

import numpy as np
import ml_dtypes

B = 256
IN_F = 1024
OUT_F = 128
KD = 16
N_CORES = 8
O_PER_CORE = OUT_F // N_CORES      # 16
OK = O_PER_CORE * KD               # 256 = 2 partition blocks of 128
O_PER_BLOCK = 128 // KD            # 8 o's per 128-partition block
IB = 8                             # batch rows per chunk
NG = B // IB                       # 32 chunks

_BF16 = ml_dtypes.bfloat16

_CACHE: dict = {}


def _build_program():
    import concourse.bacc as bacc
    import concourse.bass as bass
    import concourse.mybir as mybir
    import concourse.tile as tile

    f32 = mybir.dt.float32
    bf16 = mybir.dt.bfloat16
    AF = mybir.ActivationFunctionType
    ALU = mybir.AluOpType

    nc = bacc.Bacc("TRN2", target_bir_lowering=False, debug=False,
                   num_devices=N_CORES)

    xt_d = nc.dram_tensor("xt", [IN_F, B], bf16, kind="ExternalInput")
    tm_d = nc.dram_tensor("tmat", [IN_F, OK], bf16, kind="ExternalInput")
    sel32_d = [
        nc.dram_tensor(f"sel32_{b}", [128, 32], bf16, kind="ExternalInput")
        for b in range(2)
    ]
    selB_d = [
        nc.dram_tensor(f"selB_{b}", [128, 128], bf16, kind="ExternalInput")
        for b in range(2)
    ]
    selc_d = nc.dram_tensor("selc", [128, O_PER_CORE], bf16,
                            kind="ExternalInput")
    rows_d = [
        nc.dram_tensor(f"rows{h}", [128, NG], f32, kind="ExternalOutput")
        for h in range(2)
    ]
    cols_d = nc.dram_tensor("cols", [O_PER_CORE, B], f32,
                            kind="ExternalOutput")

    with tile.TileContext(nc) as tc:
        with (
            tc.tile_pool(name="cst", bufs=1) as cst,
            tc.tile_pool(name="big", bufs=1) as big,
            tc.tile_pool(name="mt", bufs=1) as mtp,
            tc.tile_pool(name="tmp", bufs=32) as tmpp,
            tc.tile_pool(name="ee", bufs=6) as ep,
            tc.tile_pool(name="res", bufs=1) as resp,
            tc.tile_pool(name="pp", bufs=1, space=bass.MemorySpace.PSUM) as pp,
            tc.tile_pool(name="pn", bufs=3, space=bass.MemorySpace.PSUM) as pn,
            tc.tile_pool(name="pc", bufs=1, space=bass.MemorySpace.PSUM) as pc,
        ):
            # ---------- load inputs ----------
            xt_sb = big.tile([128, IN_F // 128, B], bf16, tag="xt")
            tm_sb = big.tile([128, IN_F // 128, OK], bf16, tag="tm")
            sel32_sb = [cst.tile([128, 32], bf16, tag=f"sel32_{b}", name=f"sel32_{b}")
                        for b in range(2)]
            selB_sb = [cst.tile([128, 128], bf16, tag=f"selB_{b}",
                                name=f"selB_{b}")
                       for b in range(2)]
            selc_sb = cst.tile([128, O_PER_CORE], bf16, tag="selc")
            xt_r = xt_d[:].rearrange("(c p) i -> p c i", p=128)
            tm_r = tm_d[:].rearrange("(c p) o -> p c o", p=128)
            nc.sync.dma_start(xt_sb[:, :4, :], xt_r[:, :4, :])
            nc.scalar.dma_start(xt_sb[:, 4:, :], xt_r[:, 4:, :])
            nc.gpsimd.dma_start(tm_sb[:, :4, :], tm_r[:, :4, :])
            nc.gpsimd.dma_start(tm_sb[:, 4:, :], tm_r[:, 4:, :])
            for b in range(2):
                nc.gpsimd.dma_start(sel32_sb[b][:], sel32_d[b][:])
                nc.gpsimd.dma_start(selB_sb[b][:], selB_d[b][:])
            nc.gpsimd.dma_start(selc_sb[:], selc_d[:])

            # ---------- projection: Mt[ok, i] ----------
            # mtb: bf16 working copy; mt32: exact fp32 upcast of the SAME
            # bf16 values so the per-partition scalar (must be fp32) matches
            # the streamed operand bit-for-bit -> norm[i,i] == 0 exactly.
            mtb = []
            mt32 = []
            mt32h = []
            for b in range(2):
                pm = pp.tile([128, 512], f32, tag="pm")
                for c in range(IN_F // 128):
                    nc.tensor.matmul(
                        pm[:, :B],
                        tm_sb[:, c, b * 128:(b + 1) * 128],
                        xt_sb[:, c, :],
                        start=(c == 0),
                        stop=(c == IN_F // 128 - 1),
                    )
                mb = mtp.tile([128, B], bf16, tag=f"mtb{b}", name=f"mtb{b}")
                m3 = mtp.tile([128, B], f32, tag=f"mt32{b}", name=f"mt32{b}")
                mh = mtp.tile([128, B], f32, tag=f"mt32h{b}", name=f"mt32h{b}")
                nc.vector.tensor_copy(mb[:], pm[:, :B])
                nc.vector.tensor_copy(m3[:], mb[:])
                # exact fp32 halves of the bf16 Mt values
                nc.vector.tensor_scalar(mh[:], mb[:], 0.5, None, op0=ALU.mult)
                mtb.append(mb)
                mt32.append(m3)
                mt32h.append(mh)

            # ---------- main loop over row chunks ----------
            rows_sb = [resp.tile([128, NG], f32, tag=f"rows{h}", name=f"rows{h}_sb")
                       for h in range(2)]
            pcol = pc.tile([O_PER_CORE, 512], f32, tag="pcol")

            for g in range(NG):
                jlo = IB * g
                jw = B - jlo
                pnorm = [pn.tile([128, 512], f32, tag=f"pnorm{h}",
                                  name=f"pnorm{h}")
                         for h in range(2)]
                # psum = S_j - 2*sum_k t = norm, with t = min(Mt_j, m_i) - m_i/2:
                # the S_j term comes from one [128,128]-selector matmul per
                # block that fills all four 32-row regions at once, and the
                # t-matmuls (selector entries -2) accumulate on top.
                for h in range(2):
                    for b in range(2):
                        nc.tensor.matmul(
                            pnorm[h][:, :jw],
                            selB_sb[b][:],
                            mtb[b][:, jlo:B],
                            start=(b == 0),
                            stop=False,
                            skip_group_check=True,
                        )
                # GPSIMD takes a share of the abs-ops; its per-op cost is
                # ~linear in jw while DVE is mostly fixed overhead, so the
                # optimal share grows as jw shrinks.
                if g < 2:
                    gps = ((0, 1), (4, 0))
                elif g < 8:
                    gps = ((0, 0), (0, 1), (4, 0), (4, 1))
                elif g < 16:
                    gps = ((0, 0), (0, 1), (2, 0), (4, 0), (4, 1))
                elif g < 28:
                    gps = ((0, 0), (0, 1), (2, 0), (4, 0), (4, 1), (6, 1))
                else:
                    gps = ((0, 0), (0, 1), (2, 0), (3, 1), (4, 0), (4, 1),
                           (6, 1))
                for s in range(IB):
                    i = jlo + s
                    h, r0 = divmod(s, 4)
                    for b in range(2):
                        t = tmpp.tile([128, jw], bf16, tag="tmp", name="t")
                        teng = nc.gpsimd if (s, b) in gps else nc.vector
                        teng.tensor_scalar(
                            t[:],
                            mtb[b][:, jlo:B],
                            mt32[b][:, i:i + 1],
                            mt32h[b][:, i:i + 1],
                            op0=ALU.min,
                            op1=ALU.subtract,
                        )
                        nc.tensor.matmul(
                            pnorm[h][r0 * 32: r0 * 32 + 32, :jw],
                            sel32_sb[b][:],
                            t[:],
                            start=False,
                            stop=(r0 == 3 and b == 1),
                            tile_position=(0, r0 * 32),
                            skip_group_check=True,
                        )
                ee = []
                for h in range(2):
                    e = ep.tile([128, jw], bf16, tag=f"e{h}", name=f"e{h}")
                    nc.scalar.activation(
                        e[:],
                        pnorm[h][:, :jw],
                        AF.Exp,
                        scale=-1.0,
                        accum_out=rows_sb[h][:, g:g + 1],
                    )
                    ee.append(e)
                if g < NG - 1:
                    for h in range(2):
                        nc.tensor.matmul(
                            pcol[:, jlo + IB:B],
                            selc_sb[:],
                            ee[h][:, IB:],
                            start=(g == 0 and h == 0),
                            stop=(g == NG - 2 and h == 1),
                            skip_group_check=True,
                        )
                if g == 15:
                    # first 16 row-sum columns are final; drain them early
                    nc.sync.dma_start(rows_d[0][:, :16], rows_sb[0][:, :16])
                    nc.scalar.dma_start(rows_d[1][:, :16], rows_sb[1][:, :16])

            # ---------- finalize ----------
            cols_sb = resp.tile([O_PER_CORE, B], f32, tag="cols")
            nc.vector.memset(cols_sb[:, 0:IB], 0.0)
            nc.vector.tensor_copy(cols_sb[:, IB:B], pcol[:, IB:B])
            nc.sync.dma_start(rows_d[0][:, 16:], rows_sb[0][:, 16:])
            nc.scalar.dma_start(rows_d[1][:, 16:], rows_sb[1][:, 16:])
            nc.gpsimd.dma_start(cols_d[:], cols_sb[:])

    nc.compile()
    return nc


def get_program():
    if "nc" not in _CACHE:
        _CACHE["nc"] = _build_program()
    return _CACHE["nc"]


def make_selectors():
    p = np.arange(128)
    sel32 = []
    selB = []
    for b in range(2):
        s = np.zeros((128, 32), dtype=_BF16)
        s[p, p // KD + b * O_PER_BLOCK] = -2   # cols b*8..b*8+8 active
        sel32.append(s)
        sb = np.zeros((128, 128), dtype=_BF16)
        for r0 in range(4):
            sb[p, r0 * 32 + p // KD + b * O_PER_BLOCK] = 1
        selB.append(sb)
    selc = np.zeros((128, O_PER_CORE), dtype=_BF16)
    selc[p % 32 < 16, p[p % 32 < 16] % 32] = 1
    return sel32, selB, selc


def prepare_in_maps(x: np.ndarray, T: np.ndarray) -> list[dict[str, np.ndarray]]:
    x = np.asarray(x, dtype=np.float32)
    T = np.asarray(T, dtype=np.float32)
    xt = np.ascontiguousarray(x.T).astype(_BF16)
    sel32, selB, selc = make_selectors()
    in_maps = []
    for c in range(N_CORES):
        tm = np.ascontiguousarray(
            T[:, c * O_PER_CORE:(c + 1) * O_PER_CORE, :].reshape(IN_F, OK)
        ).astype(_BF16)
        in_maps.append({
            "xt": xt, "tmat": tm,
            "sel32_0": sel32[0], "sel32_1": sel32[1],
            "selB_0": selB[0], "selB_1": selB[1], "selc": selc,
        })
    return in_maps


def assemble_output(x: np.ndarray,
                    results: list[dict[str, np.ndarray]]) -> np.ndarray:
    x = np.asarray(x, dtype=np.float32)
    o_b = np.empty((B, OUT_F), dtype=np.float32)
    for c in range(N_CORES):
        cols = np.asarray(results[c]["cols"], dtype=np.float32)  # [16, B]
        R = np.empty((B, O_PER_CORE), dtype=np.float32)
        for h in range(2):
            rows = np.asarray(results[c][f"rows{h}"], dtype=np.float32)
            # rows[(s%4)*32 + o, g] -> R[i = g*8 + h*4 + s%4, o]
            r4 = rows.reshape(4, 32, NG)[:, :O_PER_CORE, :]   # [s4, o, g]
            idx = (np.arange(NG)[:, None] * IB + h * 4 +
                   np.arange(4)[None, :]).reshape(-1)
            R[idx, :] = r4.transpose(2, 0, 1).reshape(NG * 4, O_PER_CORE)
        o_b[:, c * O_PER_CORE:(c + 1) * O_PER_CORE] = R + cols.T - 1.0
    return np.concatenate([x, o_b], axis=1)


def kernel(x: np.ndarray, T: np.ndarray) -> np.ndarray:
    from concourse.bass_utils import run_bass_kernel_spmd

    nc = get_program()
    in_maps = prepare_in_maps(x, T)
    res = run_bass_kernel_spmd(nc, in_maps, list(range(N_CORES)))
    return assemble_output(x, res.results)
